# revision 1
# baseline (speedup 1.0000x reference)
"""ArcFace loss (B=8192, D=512, C=500000) on 8 TRN2 NeuronCores.

v2 strategy - shard the softmax REDUCTION, not the columns:
  - Host routes W rows into 8 disjoint shards (C/8 rows each) so core k's
    shard holds the centers for batch rows [k*B/8, (k+1)*B/8)  (labels are
    distinct).  Upload = exactly one copy of W + a replicated bf16 x.
  - Core k gathers + L2-normalizes its 1024 centers (fp32), PE-transposes
    them to cnT (bf16) - its 1024 columns of the cosine matrix stay LOCAL;
    there is no center AllGather at all.
  - x is shipped bf16 and DMA-transposed on device to xT [D, B]; the
    cos-matmul runs over ALL 8192 rows x 1024 local columns with both
    operands SBUF-resident - zero DMA inside the main loop.
  - Row norms 1/||x_i|| are folded into the ScalarE Exp as a per-partition
    scale AP: exp((S/||x_i||) * (x_i . cn_j)), with the row-sum accumulated
    by accum_out.  The B x B matrix never exists anywhere.
  - Each core's partial sum-exp [B] (32 KB) goes through one small
    AllReduce(add); each core then indirect-gathers the rows it owns,
    applies the arcface-margin diagonal corrections
    (sumexp += exp(S*t') - exp(S*t), t' = t*cos(M) - sqrt(1-t^2)*sin(M)),
    takes Ln, and emits its scalar partial loss:
      loss = (1/B) sum_i [A1*lse_i - A2*t'_i],
      A1 = (1-eps) + eps*B/C,  A2 = (1-eps)*S.
    The (eps/C)*sum_j logits_ij term is dropped - its contribution (~7e-6
    absolute on a loss of ~39) is below fp32 resolution of the result.
  - Host sums the 8 partials and divides by B.
"""

import sys

if "/opt/trn_rl_repo" not in sys.path:
    sys.path.insert(0, "/opt/trn_rl_repo")

import math
from contextlib import ExitStack

import numpy as np
import ml_dtypes

import concourse.bacc as bacc
import concourse.bass as bass
import concourse.tile as tile
from concourse import mybir
from concourse.bass_utils import run_bass_kernel_spmd
from concourse.masks import make_identity

F32 = mybir.dt.float32
BF16 = mybir.dt.bfloat16
FP8 = mybir.dt.float8e4
I32 = mybir.dt.int32
P = 128

# problem constants (hardcoded; kernel.py must be self-contained)
B, D, C = 8192, 512, 500000
NCORES = 8
MARGIN, S_SCALE, EPS = 0.5, 64.0, 0.1


def build_nc(b, d, csh, ncores, s_scale, margin, eps, c_total):
    """Build + compile the (identical-on-every-core) bass graph."""
    bl = b // ncores          # local columns (own batch rows)
    nt = bl // P              # own row tiles
    nm = b // P               # global row tiles
    kc_n = d // P             # contraction chunks
    nbc = min(512, bl)        # matmul moving free dim
    ncb = bl // nbc           # column sub-blocks (matmuls per psum row-block)
    mg = 4 if nm % 4 == 0 else 1   # row-tile group size for the norm stream
    ng = nm // mg
    gp = max(nt, 2)           # gather rows from ar_out (pad to >=2)
    a1 = (1.0 - eps) + eps * b / c_total
    a2 = (1.0 - eps) * s_scale
    cos_m = float(math.cos(margin))
    sin_m = float(math.sin(margin))

    nc = bacc.Bacc(
        "TRN2",
        target_bir_lowering=False,
        debug=False,
        enable_asserts=False,
        num_devices=ncores,
    )
    w_ext = nc.dram_tensor("w", [csh, d], F32, kind="ExternalInput")
    xb_ext = nc.dram_tensor("xb", [b, d], BF16, kind="ExternalInput")
    xt8_ext = nc.dram_tensor("xt8", [d, b], FP8, kind="ExternalInput")
    idx_ext = nc.dram_tensor("idx", [P, nt], I32, kind="ExternalInput")
    idxx_ext = nc.dram_tensor("idxx", [P, nt], I32, kind="ExternalInput")
    out_ext = nc.dram_tensor("out", [1, 1], F32, kind="ExternalOutput")

    with tile.TileContext(nc) as tc:
        es = ExitStack()
        const = es.enter_context(tc.tile_pool(name="const", bufs=1))
        small = es.enter_context(tc.tile_pool(name="small", bufs=3))
        strm = es.enter_context(tc.tile_pool(name="strm", bufs=4))
        dram = es.enter_context(tc.tile_pool(name="dram", bufs=1, space="DRAM"))
        tp_es = ExitStack()
        tp_psum = tp_es.enter_context(tc.tile_pool(name="tp_psum", bufs=6, space="PSUM"))

        ident = const.tile([P, P], F32, name="ident")
        make_identity(nc, ident[:])

        idx_sb = const.tile([P, nt], I32, name="idx_sb")
        nc.sync.dma_start(out=idx_sb[:], in_=idx_ext[:, :])
        idxx_sb = const.tile([P, nt], I32, name="idxx_sb")
        nc.sync.dma_start(out=idxx_sb[:], in_=idxx_ext[:, :])

        cent = const.tile([P, nt * d], F32, name="cent")
        scr32 = const.tile([P, nt * d], F32, name="scr32")
        cn = const.tile([P, nt * d], F32, name="cn")
        cnT = const.tile([P, kc_n * bl], FP8, name="cnT")
        xT = const.tile([P, kc_n * b], FP8, name="xT")
        xg = const.tile([P, nt * d], BF16, name="xg")
        tcol = const.tile([P, nt], F32, name="tcol")
        nsqall = const.tile([P, nm], F32, name="nsqall")
        sescall = const.tile([P, nm], F32, name="sescall")
        separt = const.tile([P, nm], F32, name="separt")

        ar_in = dram.tile([nm, P], F32, name="ar_in")
        ar_out = dram.tile([nm // ncores, P], F32, name="ar_out")

        def chunk(tile_, t):
            return tile_[:, t * d : (t + 1) * d]

        # ---- center path: gather -> normalize (fp32) -> transpose (fp8) ----
        hp = tc.high_priority()
        hp.__enter__()
        for t in range(nt):
            nc.gpsimd.indirect_dma_start(
                out=chunk(cent, t), out_offset=None, in_=w_ext[:, :],
                in_offset=bass.IndirectOffsetOnAxis(ap=idx_sb[:, t : t + 1], axis=0),
            )
        ssqc = small.tile([P, nt], F32, name="ssqc")
        for t in range(nt):
            sqc8 = strm.tile([P, d], F32, name="sqc8")
            nc.scalar.activation(
                out=sqc8[:], in_=chunk(cent, t),
                func=mybir.ActivationFunctionType.Square,
                accum_out=ssqc[:, t : t + 1],
            )
        nrmc = small.tile([P, nt], F32, name="nrmc")
        nc.scalar.sqrt(nrmc[:], ssqc[:])
        recc = small.tile([P, nt], F32, name="recc")
        nc.vector.reciprocal(recc[:], nrmc[:])
        for t in range(nt):
            nc.scalar.mul(out=chunk(cn, t), in_=chunk(cent, t), mul=recc[:, t : t + 1])
        for t in range(nt):
            for kk in range(kc_n):
                pt = tp_psum.tile([P, P], F32, name="ptc")
                nc.tensor.transpose(
                    out=pt[:], in_=cn[:, t * d + kk * P : t * d + (kk + 1) * P],
                    identity=ident[:],
                )
                nc.vector.tensor_copy(
                    out=cnT[:, kk * bl + t * P : kk * bl + (t + 1) * P], in_=pt[:]
                )
        hp.__exit__(None, None, None)

        # ---- xT: load the host-pretransposed fp8 x ----
        for kk in range(kc_n):
            nc.sync.dma_start(
                out=xT[:, kk * b : (kk + 1) * b],
                in_=xt8_ext[kk * P : (kk + 1) * P, :],
            )

        # ---- row-norm stream: DVE squares/reduces (deprioritized), sqrt in
        # quarter batches so at most one activation-table swap lands inside
        # the Exp phase. ----
        qn = min(4, ng) if nm % 4 == 0 else 1
        qs = nm // qn
        gq = ng // qn             # stream groups per quarter
        with tc.high_priority(offset=-1000000):
            for g in range(ng):
                rowx = strm.tile([P, mg * d], BF16, name="rowx")
                nc.sync.dma_start(
                    out=rowx[:].rearrange("p (t c) -> p t c", c=d),
                    in_=xb_ext[g * mg * P : (g + 1) * mg * P, :].rearrange(
                        "(t p) c -> p t c", p=P
                    ),
                )
                sqr = strm.tile([P, mg * d], BF16, name="sqr")
                nc.vector.tensor_tensor(
                    out=sqr[:], in0=rowx[:], in1=rowx[:], op=mybir.AluOpType.mult
                )
                nc.vector.tensor_reduce(
                    out=nsqall[:, g * mg : (g + 1) * mg],
                    in_=sqr[:].rearrange("p (t c) -> p t c", c=d),
                    axis=mybir.AxisListType.X, op=mybir.AluOpType.add,
                )
                if (g + 1) % gq == 0:
                    qq = g // gq
                    nrmq = small.tile([P, qs], F32, name="nrmq")
                    nc.scalar.sqrt(nrmq[:], nsqall[:, qq * qs : (qq + 1) * qs])
                    recq = small.tile([P, qs], F32, name="recq")
                    nc.vector.reciprocal(recq[:], nrmq[:])
                    nc.vector.tensor_scalar_mul(
                        out=sescall[:, qq * qs : (qq + 1) * qs], in0=recq[:],
                        scalar1=s_scale,
                    )

        # ---- own-row path: gather x rows, fp32 normalize, margin terms ----
        for t in range(nt):
            nc.gpsimd.indirect_dma_start(
                out=chunk(xg, t), out_offset=None, in_=xb_ext[:, :],
                in_offset=bass.IndirectOffsetOnAxis(ap=idxx_sb[:, t : t + 1], axis=0),
            )
        nc.scalar.activation(
            out=scr32[:], in_=xg[:], func=mybir.ActivationFunctionType.Square
        )
        ssqg = small.tile([P, nt], F32, name="ssqg")
        nc.vector.tensor_reduce(
            out=ssqg[:], in_=scr32[:].rearrange("p (t c) -> p t c", c=d),
            axis=mybir.AxisListType.X, op=mybir.AluOpType.add,
        )
        nrmg2 = small.tile([P, nt], F32, name="nrmg2")
        nc.scalar.sqrt(nrmg2[:], ssqg[:])
        recg2 = small.tile([P, nt], F32, name="recg2")
        nc.vector.reciprocal(recg2[:], nrmg2[:])
        # t_i = (x_i . cn_i) / ||x_i||  (normalize folded into the reduce)
        nc.vector.tensor_tensor(
            out=scr32[:], in0=xg[:], in1=cn[:], op=mybir.AluOpType.mult
        )
        traw = small.tile([P, nt], F32, name="traw")
        nc.vector.tensor_reduce(
            out=traw[:], in_=scr32[:].rearrange("p (t c) -> p t c", c=d),
            axis=mybir.AxisListType.X, op=mybir.AluOpType.add,
        )
        nc.vector.tensor_tensor(
            out=tcol[:], in0=traw[:], in1=recg2[:], op=mybir.AluOpType.mult
        )
        tsq = const.tile([P, nt], F32, name="tsq")
        nc.vector.tensor_tensor(
            out=tsq[:], in0=tcol[:], in1=tcol[:], op=mybir.AluOpType.mult
        )
        s1m = const.tile([P, nt], F32, name="s1m")
        nc.scalar.activation(
            out=s1m[:], in_=tsq[:], func=mybir.ActivationFunctionType.Sqrt,
            bias=1.0, scale=-1.0,
        )  # sqrt(1 - t^2)
        tpa = const.tile([P, nt], F32, name="tpa")
        nc.vector.tensor_scalar_mul(out=tpa[:], in0=tcol[:], scalar1=cos_m)
        tpb = const.tile([P, nt], F32, name="tpb")
        nc.vector.tensor_scalar_mul(out=tpb[:], in0=s1m[:], scalar1=sin_m)
        tpcol = const.tile([P, nt], F32, name="tpcol")
        nc.vector.tensor_tensor(
            out=tpcol[:], in0=tpa[:], in1=tpb[:], op=mybir.AluOpType.subtract
        )
        expt = const.tile([P, nt], F32, name="expt")
        nc.scalar.activation(
            out=expt[:], in_=tcol[:], func=mybir.ActivationFunctionType.Exp,
            scale=s_scale,
        )
        exptp = const.tile([P, nt], F32, name="exptp")
        nc.scalar.activation(
            out=exptp[:], in_=tpcol[:], func=mybir.ActivationFunctionType.Exp,
            scale=s_scale,
        )
        ecorr = const.tile([P, nt], F32, name="ecorr")
        nc.vector.tensor_tensor(
            out=ecorr[:], in0=exptp[:], in1=expt[:], op=mybir.AluOpType.subtract
        )

        tp_es.close()

        # ---- main loop: all 8192 rows x local bl columns, SBUF-resident ----
        with (
            tc.tile_pool(name="expp", bufs=6) as expp,
            tc.tile_pool(name="mm_psum", bufs=4, space="PSUM") as mm_psum,
        ):
            assert kc_n % 2 == 0
            xT3 = xT[:].rearrange("p (k q) -> p k q", q=b)
            cnT3 = cnT[:].rearrange("p (k q) -> p k q", q=bl)
            for m in range(nm):
                ps = mm_psum.tile([P, bl], F32, name="mmblk")
                for kg in range(kc_n // 2):
                    for h in range(ncb):
                        nc.tensor.matmul(
                            out=ps[:, h * nbc : (h + 1) * nbc],
                            lhsT=xT3[:, 2 * kg : 2 * kg + 2, m * P : (m + 1) * P],
                            rhs=cnT3[:, 2 * kg : 2 * kg + 2, h * nbc : (h + 1) * nbc],
                            start=(kg == 0),
                            stop=(kg == kc_n // 2 - 1),
                            perf_mode=mybir.MatmulPerfMode.DoubleRow,
                        )
                scr = expp.tile([P, bl], BF16, name="expscr")
                nc.scalar.activation(
                    out=scr[:], in_=ps[:],
                    func=mybir.ActivationFunctionType.Exp,
                    scale=sescall[:, m : m + 1],
                    accum_out=separt[:, m : m + 1],
                )

        # ---- AllReduce the partial sum-exp (32 KB) ----
        fin_psum = es.enter_context(tc.tile_pool(name="fin_psum", bufs=1, space="PSUM"))
        seT = fin_psum.tile([nm, P], F32, name="seT") if nm <= P else None
        assert nm <= P
        nc.tensor.transpose(out=seT[:], in_=separt[:], identity=ident[:])
        seTs = const.tile([nm, P], F32, name="seTs")
        nc.vector.tensor_copy(out=seTs[:], in_=seT[:])
        nc.sync.dma_start(out=ar_in[:, :], in_=seTs[:])
        nc.gpsimd.collective_compute(
            "ReduceScatter",
            mybir.AluOpType.add,
            replica_groups=[list(range(ncores))],
            ins=[ar_in[:].opt()],
            outs=[ar_out[:].opt()],
        )

        # ---- rank k's scatter slice IS its own rows; apply corrections ----
        seg = const.tile([nt, P], F32, name="seg")
        nc.sync.dma_start(out=seg[:], in_=ar_out[:, :])
        segT = fin_psum.tile([P, nt], F32, name="segT")
        nc.tensor.transpose(out=segT[:], in_=seg[:], identity=ident[:nt, :nt])
        se_own = const.tile([P, nt], F32, name="se_own")
        nc.vector.tensor_copy(out=se_own[:], in_=segT[:, :nt])

        secor2 = const.tile([P, nt], F32, name="secor2")
        nc.vector.tensor_tensor(
            out=secor2[:], in0=se_own[:], in1=ecorr[:], op=mybir.AluOpType.add
        )
        lse = const.tile([P, nt], F32, name="lse")
        nc.scalar.activation(
            out=lse[:], in_=secor2[:], func=mybir.ActivationFunctionType.Ln
        )
        ra = const.tile([P, nt], F32, name="ra")
        nc.vector.tensor_scalar_mul(out=ra[:], in0=lse[:], scalar1=a1)
        rb = const.tile([P, nt], F32, name="rb")
        nc.vector.tensor_scalar_mul(out=rb[:], in0=tpcol[:], scalar1=a2)
        rterm = const.tile([P, nt], F32, name="rterm")
        nc.vector.tensor_tensor(
            out=rterm[:], in0=ra[:], in1=rb[:], op=mybir.AluOpType.subtract
        )
        rsum = const.tile([P, 1], F32, name="rsum")
        nc.vector.tensor_reduce(
            out=rsum[:], in_=rterm[:], axis=mybir.AxisListType.X,
            op=mybir.AluOpType.add,
        )
        ones = const.tile([P, 1], F32, name="ones")
        nc.vector.memset(ones[:], 1.0)
        fin = fin_psum.tile([1, 1], F32, name="fin")
        nc.tensor.matmul(out=fin[:], lhsT=ones[:], rhs=rsum[:], start=True, stop=True)
        res = const.tile([1, 1], F32, name="res")
        nc.vector.tensor_copy(out=res[:], in_=fin[:])
        nc.sync.dma_start(out=out_ext[:, :], in_=res[:])

        es.close()

    nc.compile()
    return nc


def make_in_maps(x, labels, W, ncores=NCORES):
    """Host-side sharding: route W rows so core k's shard holds the centers
    for batch rows [k*bl, (k+1)*bl).  Returns per-core input dicts."""
    b, d = x.shape
    c = W.shape[0]
    bl = b // ncores
    csh = c // ncores
    nt = bl // P
    gp = max(nt, 2)
    labels = np.asarray(labels).astype(np.int64)
    assert len(np.unique(labels)) == b, "routing assumes distinct labels"

    xb = np.ascontiguousarray(x.astype(ml_dtypes.bfloat16))
    xt8 = np.ascontiguousarray(x.T.astype(ml_dtypes.float8_e4m3))

    owner = np.full(c, -1, np.int8)
    for k in range(ncores):
        owner[labels[k * bl : (k + 1) * bl]] = k
    free_rows = np.flatnonzero(owner < 0)
    pos = 0
    in_maps = []
    p_ar = np.arange(P, dtype=np.int32)
    t_ar = np.arange(nt, dtype=np.int32)
    for k in range(ncores):
        mine = np.flatnonzero(owner == k)
        need = csh - len(mine)
        extra = free_rows[pos : pos + need]
        pos += need
        rows = np.sort(np.concatenate([mine, extra]))
        lab = labels[k * bl : (k + 1) * bl]
        loc = np.searchsorted(rows, lab)
        assert np.array_equal(rows[loc], lab)
        idx = np.ascontiguousarray(loc.astype(np.int32).reshape(nt, P).T)
        idxx = np.ascontiguousarray(
            (k * bl + t_ar[None, :] * P + p_ar[:, None]).astype(np.int32)
        )
        in_maps.append(
            {
                "w": np.ascontiguousarray(W[rows]),
                "xb": xb,
                "xt8": xt8,
                "idx": idx,
                "idxx": idxx,
            }
        )
    return in_maps


_compiled_nc = None


def get_compiled():
    global _compiled_nc
    if _compiled_nc is None:
        _compiled_nc = build_nc(
            B, D, C // NCORES, NCORES, S_SCALE, MARGIN, EPS, C
        )
    return _compiled_nc


def run(x, labels, W, trace=False, trace_cores=None):
    nc = get_compiled()
    in_maps = make_in_maps(
        np.asarray(x, dtype=np.float32), labels, np.asarray(W, dtype=np.float32)
    )
    res = run_bass_kernel_spmd(
        nc,
        in_maps,
        core_ids=list(range(NCORES)),
        trace=trace,
        trace_cores=trace_cores,
    )
    total = sum(float(r["out"][0, 0]) for r in res.results)
    return np.float32(total / B), res


def kernel(**inputs):
    loss, _ = run(inputs["x"], inputs["labels"], inputs["W"])
    return loss



# revision 6
# speedup vs baseline: 1.4181x; 1.4181x over previous
"""ArcFace loss (B=8192, D=512, C=500000) on 8 TRN2 NeuronCores.

v3 strategy - column-sharded softmax reduction (as v2) with a lean prefix
and a single activation table:
  - Host routes exactly the 1024 centers core k needs (rows W[labels] for
    its batch slice), pre-tiled to [128, 8*512] fp32 - the on-device
    indirect gather is replaced by one contiguous DMA (2 MB).
  - Own x rows ship as a pre-tiled [128, 8*512] bf16 slice (1 MB);
    x.T ships replicated as fp8 [512, 8192] for the PE cos-matmul.
  - All rsqrt/sqrt needs use exp(+-0.5*ln(.)) so the WHOLE program runs
    off one activation table (natural_log_exp_and_others: square/exp/ln/
    copy) - zero ACT_TABLE_LOAD swaps in the main loop (v2 had 12).
  - Row norms ||x_i|| for the Exp row-scale are computed locally (no
    cross-core dependency before the final collective, so start skew is
    absorbed once): xb bf16 streams in 16 groups; DVE squares+reduces at
    bf16; quarters convert to S/||x_i|| via one Ln + one biased Exp.
  - Main loop: 64 row tiles x [128 x 1024] fp8 DoubleRow matmuls, ScalarE
    Exp with per-partition scale and accum_out row-sums. The B x B matrix
    never exists anywhere.
  - One 32 KB ReduceScatter(add) of the partial sum-exp; rank k's scatter
    slice is its own rows; margin corrections + Ln + partial loss as v2.
  - Host sums the 8 partial losses and divides by B.
"""

import sys

if "/opt/trn_rl_repo" not in sys.path:
    sys.path.insert(0, "/opt/trn_rl_repo")

import math
from contextlib import ExitStack

import numpy as np
import ml_dtypes

import concourse.bacc as bacc
import concourse.bass as bass
import concourse.tile as tile
from concourse import mybir
from concourse.bass_utils import run_bass_kernel_spmd
from concourse.masks import make_identity

F32 = mybir.dt.float32
BF16 = mybir.dt.bfloat16
FP8 = mybir.dt.float8e4
I32 = mybir.dt.int32
P = 128

# problem constants (hardcoded; kernel.py must be self-contained)
B, D, C = 8192, 512, 500000
NCORES = 8
MARGIN, S_SCALE, EPS = 0.5, 64.0, 0.1


def build_nc(b, d, ncores, s_scale, margin, eps, c_total):
    """Build + compile the (identical-on-every-core) bass graph."""
    bl = b // ncores          # local columns (own batch rows)
    nt = bl // P              # own row tiles
    nm = b // P               # global row tiles
    kc_n = d // P             # contraction chunks
    nbc = min(512, bl)        # matmul moving free dim
    ncb = bl // nbc           # column sub-blocks (matmuls per psum row-block)
    mg = 4                    # row tiles per norm-stream group
    ng = nm // mg             # norm-stream groups
    qn = 4                    # quarters for the S/||x|| conversion
    qs = nm // qn
    gq = ng // qn             # stream groups per quarter
    a1 = (1.0 - eps) + eps * b / c_total
    a2 = (1.0 - eps) * s_scale
    cos_m = float(math.cos(margin))
    sin_m = float(math.sin(margin))
    ln_s = float(math.log(s_scale))

    nc = bacc.Bacc(
        "TRN2",
        target_bir_lowering=False,
        debug=False,
        enable_asserts=False,
        num_devices=ncores,
    )
    # host-pretiled inputs: [128, k*512] where partition p, chunk t holds
    # batch row t*128+p of the respective slice
    cent_ext = nc.dram_tensor("cent", [P, nt * d], F32, kind="ExternalInput")
    xsl_ext = nc.dram_tensor("xsl", [P, nt * d], BF16, kind="ExternalInput")
    xb_ext = nc.dram_tensor("xbt", [P, nm * d], BF16, kind="ExternalInput")
    xt8_ext = nc.dram_tensor("xt8", [d, b], FP8, kind="ExternalInput")
    out_ext = nc.dram_tensor("out", [1, 1], F32, kind="ExternalOutput")

    with tile.TileContext(nc) as tc:
        es = ExitStack()
        const = es.enter_context(tc.tile_pool(name="const", bufs=1))
        small = es.enter_context(tc.tile_pool(name="small", bufs=3))
        strm = es.enter_context(tc.tile_pool(name="strm", bufs=4))
        dram = es.enter_context(tc.tile_pool(name="dram", bufs=1, space="DRAM"))
        tp_es = ExitStack()
        tp_psum = tp_es.enter_context(tc.tile_pool(name="tp_psum", bufs=6, space="PSUM"))

        ident = const.tile([P, P], F32, name="ident")
        make_identity(nc, ident[:])
        lnsb = const.tile([P, 1], F32, name="lnsb")
        nc.vector.memset(lnsb[:], ln_s)

        cent = const.tile([P, nt * d], F32, name="cent")
        cn = const.tile([P, nt * d], F32, name="cn")
        cnT = const.tile([P, kc_n * bl], FP8, name="cnT")
        xT = const.tile([P, kc_n * b], FP8, name="xT")
        xsl = const.tile([P, nt * d], BF16, name="xsl")
        tcol = const.tile([P, nt], F32, name="tcol")
        nsqb = const.tile([P, nm], BF16, name="nsqb")
        sescall = const.tile([P, nm], F32, name="sescall")
        separt = const.tile([P, nm], F32, name="separt")

        ar_in = dram.tile([nm, P], F32, name="ar_in")
        ar_out = dram.tile([nm // ncores, P], F32, name="ar_out")

        def chunk(tile_, t):
            return tile_[:, t * d : (t + 1) * d]

        # ---- center path: load -> normalize (rsqrt = exp(-ln/2)) ->
        # transpose to fp8.  All ACT functions live in one table. ----
        hp = tc.high_priority()
        hp.__enter__()
        nc.sync.dma_start(out=cent[:, : nt * d // 2], in_=cent_ext[:, : nt * d // 2])
        nc.sync.dma_start(out=cent[:, nt * d // 2 :], in_=cent_ext[:, nt * d // 2 :])
        ssqc = small.tile([P, nt], F32, name="ssqc")
        for t in range(nt):
            sqc8 = strm.tile([P, d], F32, name="sqc8")
            nc.scalar.activation(
                out=sqc8[:], in_=chunk(cent, t),
                func=mybir.ActivationFunctionType.Square,
                accum_out=ssqc[:, t : t + 1],
            )
        lnc = small.tile([P, nt], F32, name="lnc")
        nc.scalar.activation(
            out=lnc[:], in_=ssqc[:], func=mybir.ActivationFunctionType.Ln
        )
        recc = small.tile([P, nt], F32, name="recc")
        nc.scalar.activation(
            out=recc[:], in_=lnc[:], func=mybir.ActivationFunctionType.Exp,
            scale=-0.5,
        )  # 1/||c||
        for t in range(nt):
            nc.scalar.mul(out=chunk(cn, t), in_=chunk(cent, t), mul=recc[:, t : t + 1])
        for t in range(nt):
            for kk in range(kc_n):
                pt = tp_psum.tile([P, P], F32, name="ptc")
                nc.tensor.transpose(
                    out=pt[:], in_=cn[:, t * d + kk * P : t * d + (kk + 1) * P],
                    identity=ident[:],
                )
                nc.vector.tensor_copy(
                    out=cnT[:, kk * bl + t * P : kk * bl + (t + 1) * P], in_=pt[:]
                )
        hp.__exit__(None, None, None)

        # ---- xT: load the host-pretransposed fp8 x ----
        for kk in range(kc_n):
            nc.sync.dma_start(
                out=xT[:, kk * b : (kk + 1) * b],
                in_=xt8_ext[kk * P : (kk + 1) * P, :],
            )

        # ---- row-norm stream (deprioritized): DVE squares/reduces at bf16,
        # quarters -> S/||x_i|| via Ln + biased Exp (same ACT table). ----
        with tc.high_priority(offset=-1000000):
            for g in range(ng):
                rowx = strm.tile([P, mg * d], BF16, name="rowx")
                nc.sync.dma_start(
                    out=rowx[:], in_=xb_ext[:, g * mg * d : (g + 1) * mg * d]
                )
                sqr = strm.tile([P, mg * d], BF16, name="sqr")
                nc.vector.tensor_tensor(
                    out=sqr[:], in0=rowx[:], in1=rowx[:], op=mybir.AluOpType.mult
                )
                with nc.allow_low_precision(
                    reason="norm^2 in bf16: 0.4% rel err on ||x||^2 is ~2e-3 "
                    "on the exp row-scale, far inside the 2e-2 loss gate"
                ):
                    nc.vector.tensor_reduce(
                        out=nsqb[:, g * mg : (g + 1) * mg],
                        in_=sqr[:].rearrange("p (t c) -> p t c", c=d),
                        axis=mybir.AxisListType.X, op=mybir.AluOpType.add,
                    )
                if (g + 1) % gq == 0:
                    qq = g // gq
                    lnq = small.tile([P, qs], F32, name="lnq")
                    nc.scalar.activation(
                        out=lnq[:], in_=nsqb[:, qq * qs : (qq + 1) * qs],
                        func=mybir.ActivationFunctionType.Ln,
                    )
                    nc.scalar.activation(
                        out=sescall[:, qq * qs : (qq + 1) * qs], in_=lnq[:],
                        func=mybir.ActivationFunctionType.Exp,
                        scale=-0.5, bias=lnsb[:],
                    )  # S / ||x_i||

        # ---- own-row path: contiguous xsl slice, margin terms ----
        nc.sync.dma_start(out=xsl[:], in_=xsl_ext[:, :])
        ssqg = small.tile([P, nt], F32, name="ssqg")
        for t in range(nt):
            sqg8 = strm.tile([P, d], F32, name="sqg8")
            nc.scalar.activation(
                out=sqg8[:], in_=chunk(xsl, t),
                func=mybir.ActivationFunctionType.Square,
                accum_out=ssqg[:, t : t + 1],
            )
        lng = small.tile([P, nt], F32, name="lng")
        nc.scalar.activation(
            out=lng[:], in_=ssqg[:], func=mybir.ActivationFunctionType.Ln
        )
        recg = small.tile([P, nt], F32, name="recg")
        nc.scalar.activation(
            out=recg[:], in_=lng[:], func=mybir.ActivationFunctionType.Exp,
            scale=-0.5,
        )  # 1/||x_i|| (own rows)
        # t_i = (x_i . cn_i) / ||x_i||
        traw = small.tile([P, nt], F32, name="traw")
        for t in range(nt):
            scrd = strm.tile([P, d], F32, name="scrd")
            nc.vector.tensor_tensor(
                out=scrd[:], in0=chunk(xsl, t), in1=chunk(cn, t),
                op=mybir.AluOpType.mult,
            )
            nc.vector.tensor_reduce(
                out=traw[:, t : t + 1], in_=scrd[:],
                axis=mybir.AxisListType.X, op=mybir.AluOpType.add,
            )
        nc.vector.tensor_tensor(
            out=tcol[:], in0=traw[:], in1=recg[:], op=mybir.AluOpType.mult
        )
        tsq = const.tile([P, nt], F32, name="tsq")
        nc.vector.tensor_tensor(
            out=tsq[:], in0=tcol[:], in1=tcol[:], op=mybir.AluOpType.mult
        )
        # sqrt(1 - t^2) = exp(0.5 * ln(1 - t^2))
        l1m = const.tile([P, nt], F32, name="l1m")
        nc.scalar.activation(
            out=l1m[:], in_=tsq[:], func=mybir.ActivationFunctionType.Ln,
            scale=-1.0, bias=1.0,
        )
        s1m = const.tile([P, nt], F32, name="s1m")
        nc.scalar.activation(
            out=s1m[:], in_=l1m[:], func=mybir.ActivationFunctionType.Exp,
            scale=0.5,
        )
        tpa = const.tile([P, nt], F32, name="tpa")
        nc.vector.tensor_scalar_mul(out=tpa[:], in0=tcol[:], scalar1=cos_m)
        tpb = const.tile([P, nt], F32, name="tpb")
        nc.vector.tensor_scalar_mul(out=tpb[:], in0=s1m[:], scalar1=sin_m)
        tpcol = const.tile([P, nt], F32, name="tpcol")
        nc.vector.tensor_tensor(
            out=tpcol[:], in0=tpa[:], in1=tpb[:], op=mybir.AluOpType.subtract
        )
        expt = const.tile([P, nt], F32, name="expt")
        nc.scalar.activation(
            out=expt[:], in_=tcol[:], func=mybir.ActivationFunctionType.Exp,
            scale=s_scale,
        )
        exptp = const.tile([P, nt], F32, name="exptp")
        nc.scalar.activation(
            out=exptp[:], in_=tpcol[:], func=mybir.ActivationFunctionType.Exp,
            scale=s_scale,
        )
        ecorr = const.tile([P, nt], F32, name="ecorr")
        nc.vector.tensor_tensor(
            out=ecorr[:], in0=exptp[:], in1=expt[:], op=mybir.AluOpType.subtract
        )

        tp_es.close()

        # ---- main loop: all 8192 rows x local bl columns, SBUF-resident ----
        with (
            tc.tile_pool(name="expp", bufs=6) as expp,
            tc.tile_pool(name="mm_psum", bufs=4, space="PSUM") as mm_psum,
        ):
            assert kc_n % 2 == 0
            xT3 = xT[:].rearrange("p (k q) -> p k q", q=b)
            cnT3 = cnT[:].rearrange("p (k q) -> p k q", q=bl)
            for m in range(nm):
                ps = mm_psum.tile([P, bl], F32, name="mmblk")
                for kg in range(kc_n // 2):
                    for h in range(ncb):
                        nc.tensor.matmul(
                            out=ps[:, h * nbc : (h + 1) * nbc],
                            lhsT=xT3[:, 2 * kg : 2 * kg + 2, m * P : (m + 1) * P],
                            rhs=cnT3[:, 2 * kg : 2 * kg + 2, h * nbc : (h + 1) * nbc],
                            start=(kg == 0),
                            stop=(kg == kc_n // 2 - 1),
                            perf_mode=mybir.MatmulPerfMode.DoubleRow,
                        )
                scr = expp.tile([P, bl], BF16, name="expscr")
                nc.scalar.activation(
                    out=scr[:], in_=ps[:],
                    func=mybir.ActivationFunctionType.Exp,
                    scale=sescall[:, m : m + 1],
                    accum_out=separt[:, m : m + 1],
                )

        # ---- ReduceScatter the partial sum-exp (32 KB) ----
        fin_psum = es.enter_context(tc.tile_pool(name="fin_psum", bufs=1, space="PSUM"))
        assert nm <= P
        seT = fin_psum.tile([nm, P], F32, name="seT")
        nc.tensor.transpose(out=seT[:], in_=separt[:], identity=ident[:])
        seTs = const.tile([nm, P], F32, name="seTs")
        nc.vector.tensor_copy(out=seTs[:], in_=seT[:])
        nc.sync.dma_start(out=ar_in[:, :], in_=seTs[:])
        nc.gpsimd.collective_compute(
            "ReduceScatter",
            mybir.AluOpType.add,
            replica_groups=[list(range(ncores))],
            ins=[ar_in[:].opt()],
            outs=[ar_out[:].opt()],
        )

        # ---- rank k's scatter slice IS its own rows; apply corrections ----
        seg = const.tile([nt, P], F32, name="seg")
        nc.sync.dma_start(out=seg[:], in_=ar_out[:, :])
        segT = fin_psum.tile([P, nt], F32, name="segT")
        nc.tensor.transpose(out=segT[:], in_=seg[:], identity=ident[:nt, :nt])
        se_own = const.tile([P, nt], F32, name="se_own")
        nc.vector.tensor_copy(out=se_own[:], in_=segT[:, :nt])

        secor2 = const.tile([P, nt], F32, name="secor2")
        nc.vector.tensor_tensor(
            out=secor2[:], in0=se_own[:], in1=ecorr[:], op=mybir.AluOpType.add
        )
        lse = const.tile([P, nt], F32, name="lse")
        nc.scalar.activation(
            out=lse[:], in_=secor2[:], func=mybir.ActivationFunctionType.Ln
        )
        ra = const.tile([P, nt], F32, name="ra")
        nc.vector.tensor_scalar_mul(out=ra[:], in0=lse[:], scalar1=a1)
        rb = const.tile([P, nt], F32, name="rb")
        nc.vector.tensor_scalar_mul(out=rb[:], in0=tpcol[:], scalar1=a2)
        rterm = const.tile([P, nt], F32, name="rterm")
        nc.vector.tensor_tensor(
            out=rterm[:], in0=ra[:], in1=rb[:], op=mybir.AluOpType.subtract
        )
        rsum = const.tile([P, 1], F32, name="rsum")
        nc.vector.tensor_reduce(
            out=rsum[:], in_=rterm[:], axis=mybir.AxisListType.X,
            op=mybir.AluOpType.add,
        )
        ones = const.tile([P, 1], F32, name="ones")
        nc.vector.memset(ones[:], 1.0)
        fin = fin_psum.tile([1, 1], F32, name="fin")
        nc.tensor.matmul(out=fin[:], lhsT=ones[:], rhs=rsum[:], start=True, stop=True)
        res = const.tile([1, 1], F32, name="res")
        nc.vector.tensor_copy(out=res[:], in_=fin[:])
        nc.sync.dma_start(out=out_ext[:, :], in_=res[:])

        es.close()

    nc.compile()
    return nc


def _tile_rows(a, ntiles):
    """[ntiles*128, d] -> [128, ntiles*d] with partition p, chunk t holding
    row t*128+p."""
    d = a.shape[1]
    return np.ascontiguousarray(
        a.reshape(ntiles, P, d).transpose(1, 0, 2).reshape(P, ntiles * d)
    )


def make_in_maps(x, labels, W, ncores=NCORES):
    """Host-side sharding: core k gets exactly the centers + x rows for
    batch rows [k*bl, (k+1)*bl), plus replicated x.T fp8 and pre-tiled
    x bf16 for the row-norm stream."""
    b, d = x.shape
    bl = b // ncores
    nt = bl // P
    nm = b // P
    labels = np.asarray(labels).astype(np.int64)

    xb16 = x.astype(ml_dtypes.bfloat16)
    xbt = _tile_rows(xb16, nm)
    xt8 = np.ascontiguousarray(x.T.astype(ml_dtypes.float8_e4m3))

    in_maps = []
    for k in range(ncores):
        rows = labels[k * bl : (k + 1) * bl]
        cent = _tile_rows(W[rows].astype(np.float32), nt)
        xsl = _tile_rows(xb16[k * bl : (k + 1) * bl], nt)
        in_maps.append({"cent": cent, "xsl": xsl, "xbt": xbt, "xt8": xt8})
    return in_maps


_compiled_nc = None


def get_compiled():
    global _compiled_nc
    if _compiled_nc is None:
        _compiled_nc = build_nc(B, D, NCORES, S_SCALE, MARGIN, EPS, C)
    return _compiled_nc


def run(x, labels, W, trace=False, trace_cores=None):
    nc = get_compiled()
    in_maps = make_in_maps(
        np.asarray(x, dtype=np.float32), labels, np.asarray(W, dtype=np.float32)
    )
    res = run_bass_kernel_spmd(
        nc,
        in_maps,
        core_ids=list(range(NCORES)),
        trace=trace,
        trace_cores=trace_cores,
    )
    total = sum(float(r["out"][0, 0]) for r in res.results)
    return np.float32(total / B), res


def kernel(**inputs):
    loss, _ = run(inputs["x"], inputs["labels"], inputs["W"])
    return loss
